# revision 1
# baseline (speedup 1.0000x reference)
"""GNN message-passing (Net3D) Trainium2 kernel, 8-way SPMD.

Strategy
--------
* Sort edges by destination node, shard them across 8 cores by contiguous
  dst ranges (each core's partial segment-sums cover only its own node
  slice); an AllGather of the updated bf16 feature slices feeds the next
  layer's src-side gathers.
* Node layout is padded per *graph* (NG slots per graph, 8 graphs per
  core). Windows of 128 node slots; per-window edge-chunk counts KW_w
  (max over the 8 cores) keep the SPMD instruction stream uniform with
  ~5% edge padding. Ghost edges carry dst_local = -1.
* All matmuls run in bf16 (fp32 is 4x slower on the PE): message MLP,
  update MLP, segment-sum (one-hot selection matrix in bf16), transposes.
* dst-side features are local (edges sorted by dst), so the dst term of
  the message MLP is computed without any gather:
      Yt = featw^T @ W1b  (per window), then
      pm += Yt-contract via a DVE-built one-hot  oh[n,e] = (dstloc==n).
  Only src features are gathered (dma_gather from the AllGather table).
* Soft-edge sigmoids are batched per 4-window group so the ACT
  Silu<->Sigmoid table reload (1.3us each!) happens 2x per group instead
  of 2x per 512 edges.
* Readout: each core reduces its own 8 graphs and runs the readout MLP
  on [*,8]; one tiny [8,1]-per-core AllGather assembles the [64,1] output.
"""

import numpy as np

H = 128
G_FIX = 64
NCORES = 8
WIN = 128
import os as _os0
G_SIG = int(_os0.environ.get("K_GSIG", "4"))
F32MIN = -1.0e30


def _wrap_idxs(idx):
    """int idx [n] -> dma_gather layout [128, n/16] int16 (idx j at
    [j%16, j//16], replicated across the 8 groups of 16 partitions)."""
    n = idx.shape[0]
    assert n % 16 == 0
    t = idx.astype(np.int16).reshape(n // 16, 16).T  # [16, n/16]
    return np.tile(t, (8, 1))


def _prep(inputs):
    """Host-side graph preprocessing -> (params dict, per-core in_maps)."""
    import ml_dtypes
    bf16 = ml_dtypes.bfloat16
    f32 = np.float32

    d0 = np.ascontiguousarray(np.asarray(inputs["d"], dtype=f32))
    src = np.asarray(inputs["src"]).astype(np.int64)
    dst = np.asarray(inputs["dst"]).astype(np.int64)
    ngid = np.asarray(inputs["node_graph_id"]).astype(np.int64)
    G = int(np.asarray(inputs["num_graphs"]))
    N = ngid.shape[0]
    E = src.shape[0]
    assert G == G_FIX and G % NCORES == 0

    cnt = np.bincount(ngid, minlength=G).astype(np.int64)
    NG = int(max(-(-int(cnt.max()) // 32) * 32, 32))   # graph slot size
    NS = G * NG // NCORES                              # node slots per core
    Npad = G * NG
    NW = NS // WIN                                     # windows per core
    assert NS % WIN == 0 and Npad < 32768              # int16 gather idx

    gstart = np.zeros(G, np.int64)
    gstart[1:] = np.cumsum(cnt)[:-1]
    # node -> padded slot; real nodes spread evenly through each graph's
    # NG-slot block (equalizes per-window edge counts)
    pos_in_g = np.arange(N) - gstart[ngid]
    padpos = ngid * NG + (pos_in_g * NG) // np.maximum(cnt[ngid], 1)
    psrc = padpos[src]
    pdst = padpos[dst]

    order = np.argsort(pdst, kind="stable")
    pdst_s = pdst[order]
    psrc_s = psrc[order]
    d0_s = d0[order]

    # per-(core,window) edge counts -> shared per-window chunk counts
    NWG = Npad // WIN
    wstart = np.searchsorted(pdst_s, np.arange(NWG + 1) * WIN)
    wcnt_g = np.diff(wstart).reshape(NCORES, NW)       # [core, window]
    KWw = (-(-wcnt_g.max(axis=0) // 128)).astype(np.int64)  # per window
    KWw = np.maximum(KWw, 1)
    woff_c = np.zeros(NW + 1, np.int64)                # chunk offsets
    woff_c[1:] = np.cumsum(KWw)
    C = int(woff_c[-1])                                # chunks per core
    Ecap = C * 128
    woff_e = woff_c * 128                              # edge offsets
    KWmax = int(KWw.max())
    CAPmax = KWmax * 128

    inv = 1.0 / np.maximum(cnt, 1)
    invcntR = np.tile(inv.astype(f32), (128, 1))
    presentR = np.tile((cnt > 0).astype(f32), (128, 1))

    # AllGather chunking: one chunk per sigmoid window-group; the gather
    # table uses a chunk-major layout (all 8 cores' chunk-k rows are
    # contiguous) so each chunk's collective writes one contiguous range.
    AGG = int(_os0.environ.get("K_AGG", "2"))           # groups per AG chunk
    _agb = _os0.environ.get("K_AGB", "")
    if _agb:
        gbounds = [int(x) for x in _agb.split(",")]
        assert gbounds[0] == 0 and gbounds[-1] == NW
        assert all(b % G_SIG == 0 for b in gbounds[:-1])
    else:
        gbounds = list(range(0, NW, AGG * G_SIG)) + [NW]
    ck = [b * WIN for b in gbounds]                     # local row bounds
    NCHUNK = len(gbounds) - 1

    ck_arr = np.asarray(ck)

    def table_row(p):
        """global padded slot -> chunk-major gather-table row"""
        c = p // NS
        r = p % NS
        k = np.clip(np.searchsorted(ck_arr, r, side="right") - 1,
                    0, NCHUNK - 1)
        sz = ck_arr[k + 1] - ck_arr[k]
        return 8 * ck_arr[k] + c * sz + (r - ck_arr[k])

    # ---- weights / constants (shared by all cores) ----
    emb = np.asarray(inputs["node_embedding"], f32)
    We = np.asarray(inputs["We"], f32)
    be = np.asarray(inputs["be"], f32)
    W1 = np.asarray(inputs["mpW1"], f32)
    b1 = np.asarray(inputs["mpb1"], f32)
    W2 = np.asarray(inputs["mpW2"], f32)
    b2 = np.asarray(inputs["mpb2"], f32)
    Ws = np.asarray(inputs["mpWs"], f32)
    bs = np.asarray(inputs["mpbs"], f32)
    Wu1 = np.asarray(inputs["mpWu1"], f32)
    bu1 = np.asarray(inputs["mpbu1"], f32)
    Wu2 = np.asarray(inputs["mpWu2"], f32)
    bu2 = np.asarray(inputs["mpbu2"], f32)
    Wn1 = np.asarray(inputs["Wn1"], f32)
    bn1 = np.asarray(inputs["bn1"], f32)
    Wn2 = np.asarray(inputs["Wn2"], f32)
    bn2 = np.asarray(inputs["bn2"], f32)
    Wr1 = np.asarray(inputs["Wr1"], f32)
    br1 = np.asarray(inputs["br1"], f32)
    Wr2 = np.asarray(inputs["Wr2"], f32)
    br2 = np.asarray(inputs["br2"], f32)

    c0 = b1[0] + emb @ (W1[0, 0:128] + W1[0, 128:256])
    embPlusBu2 = emb + bu2[0]

    # wmat (f32, readout only); wmatb (bf16, everything else)
    mats = {}
    mcols = []

    def addm(name, arr):
        mats[name] = sum(a.shape[1] for a in mcols)
        mcols.append(np.ascontiguousarray(arr.astype(f32)))

    addm("Wr1a", Wr1[0:128])
    addm("Wr1b", Wr1[128:256])
    addm("Wr1c", Wr1[256:384])
    wmat = np.concatenate(mcols, axis=1)

    matsb = {}
    bcols = []
    _boff = [0]

    def addb(name, arr):
        matsb[name] = _boff[0]
        _boff[0] += arr.shape[1]
        bcols.append(np.ascontiguousarray(arr.astype(bf16)))

    for l in range(4):
        addb(f"W1a{l}", W1[l, 0:128])
        addb(f"W1b{l}", W1[l, 128:256])
        addb(f"W1c{l}", W1[l, 256:384])
        addb(f"W2{l}", W2[l])
        addb(f"Wu1{l}", Wu1[l])
        addb(f"Wu2{l}", Wu2[l])
        addb(f"Ws{l}", Ws[l])                      # [128,1]
    addb("Wn1", Wn1)
    addb("Wn2", Wn2)
    addb("identB", np.eye(128, dtype=f32))
    addb("iotaB", np.tile(np.arange(WIN, dtype=f32), (128, 1)))
    wmatb = np.concatenate(bcols, axis=1)

    fp8 = ml_dtypes.float8_e4m3
    wmat8 = np.concatenate(
        [np.ascontiguousarray(W1[l, 0:128].astype(fp8)) for l in range(4)],
        axis=1)

    vecs = {}
    vcols = []

    def addv(name, v):
        vecs[name] = len(vcols)
        vcols.append(np.asarray(v, f32).reshape(128))

    addv("be", be)
    addv("emb", emb)
    addv("c0", c0)
    addv("embPlusBu2", embPlusBu2)
    addv("bn1", bn1)
    addv("bn2", bn2)
    addv("br1", br1)
    addv("Wr2", Wr2[:, 0])
    addv("br2", np.full(128, br2[0], f32))
    addv("iotaP", np.arange(128, dtype=f32))
    for l in range(4):
        addv(f"b1{l}", b1[l])
        addv(f"b2{l}", b2[l])
        addv(f"bs{l}", np.full(128, bs[l, 0], f32))
        addv(f"bu1{l}", bu1[l])
        addv(f"bu2{l}", bu2[l])
    wvec = np.stack(vcols, axis=1)

    base_map = {
        "wmat": wmat,
        "wmat8": wmat8,
        "wmatb": wmatb,
        "wvec": np.ascontiguousarray(wvec),
        "WeT": np.ascontiguousarray(We.astype(bf16)),
    }

    # ---- per-core data ----
    in_maps = []
    for c in range(NCORES):
        lo = c * NS
        src_idx = np.zeros(Ecap, np.int64)
        dloc = np.full(Ecap, -1.0, f32)
        d0T = np.zeros((3, Ecap), bf16)
        for w in range(NW):
            gw = c * NW + w
            a, b = wstart[gw], wstart[gw + 1]
            k = b - a
            off = int(woff_e[w])
            assert k <= KWw[w] * 128
            src_idx[off:off + k] = table_row(psrc_s[a:b])
            dloc[off:off + k] = (pdst_s[a:b] - (lo + w * WIN)).astype(f32)
            d0T[:, off:off + k] = d0_s[a:b].T
        occ = np.zeros(Npad, f32)
        occ[padpos] = 1.0
        vmask = occ[lo:lo + NS].reshape(1, NS)

        m = dict(base_map)
        m.update({
            "srcW": _wrap_idxs(src_idx),
            "dstloc": np.ascontiguousarray(dloc.reshape(C, 128).T),
            "dstlocB": np.ascontiguousarray(
                np.tile(dloc.astype(bf16), (128, 1))),
            "d0T": d0T,
            "vmaskR": vmask,
            "invcnt8": np.ascontiguousarray(invcntR[:, c * 8:(c + 1) * 8]),
            "present8": np.ascontiguousarray(presentR[:, c * 8:(c + 1) * 8]),
        })
        in_maps.append(m)

    params = dict(NG=NG, NS=NS, Npad=Npad, NW=NW, C=C, Ecap=Ecap, G=G,
                  KWw=[int(x) for x in KWw], woff_c=[int(x) for x in woff_c],
                  KWmax=KWmax, CAPmax=CAPmax, AGR=ck, NCHUNK=NCHUNK,
                  AGB=gbounds,
                  mats=mats, vecs=vecs, matsb=matsb,
                  wmat_cols=wmat.shape[1], wvec_cols=wvec.shape[1],
                  wmatb_cols=wmatb.shape[1])
    return params, in_maps


def _build(P):
    import concourse.bacc as bacc
    import concourse.mybir as mybir
    import concourse.tile as tile

    f32 = mybir.dt.float32
    bf16 = mybir.dt.bfloat16
    i16 = mybir.dt.int16
    AF = mybir.ActivationFunctionType
    OP = mybir.AluOpType
    RG = [list(range(NCORES))]

    NS, NW, C, Ecap, G, NG = (P["NS"], P["NW"], P["C"], P["Ecap"],
                              P["G"], P["NG"])
    KWw, woff_c = P["KWw"], P["woff_c"]
    KWmax, CAPmax = P["KWmax"], P["CAPmax"]
    ZCAP = G_SIG * KWmax       # z cols per sigmoid group (upper bound)

    import os
    NL = int(os.environ.get("K_NLAYERS", "4"))
    FP8AG = os.environ.get("K_FP8AG") == "1"
    AGG = int(os.environ.get("K_AGG", "2"))
    AGB = P["AGB"]
    PD = (AGG + 1) * G_SIG           # gather prefetch distance (windows)
    NO_GATHER = os.environ.get("K_NO_GATHER") == "1"
    NO_AG = os.environ.get("K_NO_AG") == "1"
    nc = bacc.Bacc("TRN2", target_bir_lowering=False, debug=False,
                   num_devices=NCORES)

    t_wmat = nc.dram_tensor("wmat", [128, P["wmat_cols"]], f32, kind="ExternalInput")
    t_wvec = nc.dram_tensor("wvec", [128, P["wvec_cols"]], f32, kind="ExternalInput")
    t_wmatb = nc.dram_tensor("wmatb", [128, P["wmatb_cols"]], bf16, kind="ExternalInput")
    fp8 = mybir.dt.float8e4
    agdt = fp8 if FP8AG else bf16
    t_wmat8 = nc.dram_tensor("wmat8", [128, 512], fp8, kind="ExternalInput")
    t_We = nc.dram_tensor("WeT", [3, 128], bf16, kind="ExternalInput")
    t_inv8 = nc.dram_tensor("invcnt8", [128, 8], f32, kind="ExternalInput")
    t_pres8 = nc.dram_tensor("present8", [128, 8], f32, kind="ExternalInput")
    t_srcW = nc.dram_tensor("srcW", [128, Ecap // 16], i16, kind="ExternalInput")
    t_dstloc = nc.dram_tensor("dstloc", [128, C], f32, kind="ExternalInput")
    t_dstlocB = nc.dram_tensor("dstlocB", [128, Ecap], bf16, kind="ExternalInput")
    t_d0T = nc.dram_tensor("d0T", [3, Ecap], bf16, kind="ExternalInput")
    t_vmask = nc.dram_tensor("vmaskR", [1, NS], f32, kind="ExternalInput")
    t_out = nc.dram_tensor("out", [G, 1], f32, kind="ExternalOutput")

    with tile.TileContext(nc) as tc:
        with (
            tc.tile_pool(name="sbc", bufs=1) as sbc,
            tc.tile_pool(name="sbp", bufs=1) as sbp,      # persistent feats
            tc.tile_pool(name="sbg", bufs=int(os.environ.get("K_SBG", "15"))) as sbg,      # gather landing
            tc.tile_pool(name="sbb", bufs=int(os.environ.get("K_SBBB", "3"))) as sbb,      # dstlocB stream
            tc.tile_pool(name="sbw", bufs=int(os.environ.get("K_SBW", "3"))) as sbw,      # working tiles
            tc.tile_pool(name="sbme", bufs=int(os.environ.get("K_SBME", "5"))) as sbme,    # per-window msgE
            tc.tile_pool(name="sbs", bufs=int(os.environ.get("K_SBS", "4"))) as sbs,      # S / one-hot tiles
            tc.tile_pool(name="ps_mm", bufs=3, space="PSUM") as ps_mm,
            tc.tile_pool(name="ps_me", bufs=1, space="PSUM") as ps_me,
            tc.tile_pool(name="ps_z", bufs=1, space="PSUM") as ps_z,
            tc.tile_pool(name="ps_ms", bufs=2, space="PSUM") as ps_ms,
            tc.tile_pool(name="ps_sm", bufs=1, space="PSUM") as ps_sm,
            tc.tile_pool(name="dram", bufs=1, space="DRAM") as dram,
        ):
            # ---- constants ----
            wmat = sbc.tile([128, P["wmat_cols"]], f32, tag="wmat")
            nc.sync.dma_start(wmat[:], t_wmat[:])
            wvec = sbc.tile([128, P["wvec_cols"]], f32, tag="wvec")
            nc.sync.dma_start(wvec[:], t_wvec[:])
            wmatb = sbc.tile([128, P["wmatb_cols"]], bf16, tag="wmatb")
            nc.sync.dma_start(wmatb[:], t_wmatb[:])
            wmat8 = sbc.tile([128, 512], fp8, tag="wmat8")
            nc.sync.dma_start(wmat8[:], t_wmat8[:])
            WeT = sbc.tile([3, 128], bf16, tag="WeT")
            nc.sync.dma_start(WeT[:], t_We[:])
            invR8 = sbc.tile([128, 8], f32, tag="invR8")
            nc.sync.dma_start(invR8[:], t_inv8[:])
            presR8 = sbc.tile([128, 8], f32, tag="presR8")
            nc.sync.dma_start(presR8[:], t_pres8[:])
            srcW = sbc.tile([128, Ecap // 16], i16, tag="srcW")
            nc.sync.dma_start(srcW[:], t_srcW[:])
            dstloc = sbc.tile([128, C], f32, tag="dstloc")
            nc.sync.dma_start(dstloc[:], t_dstloc[:])
            onesP = sbc.tile([1, 128], f32, tag="onesP")
            nc.vector.memset(onesP[:], 1.0)

            def W(name):
                o = P["mats"][name]
                return wmat[:, o:o + 128]

            def Wb(name, n=128):
                o = P["matsb"][name]
                return wmatb[:, o:o + n]

            def V(name):
                o = P["vecs"][name]
                return wvec[:, o:o + 1]

            def W8(l):
                return wmat8[:, l * 128:(l + 1) * 128]

            identB = Wb("identB")
            iotaTb = Wb("iotaB", WIN)

            tc.strict_bb_all_engine_barrier()

            # ---- DRAM work buffers ----
            dT = [dram.tile([128, Ecap], bf16, tag=f"dT{i}", name=f"dT{i}")
                  for i in range(2)]
            ag_in = [dram.tile([NS, 128], agdt, tag=f"agin{l}",
                               name=f"agin{l}") for l in range(3)]
            NCH = P["NCHUNK"]
            AGR = P["AGR"]
            # CoreSim only supports a single writer per Shared tensor, so the
            # local-sim path (K_SIMAG=1) routes each AllGather chunk through
            # its own Shared tensor plus a unify copy on the ACT HWDGE queue.
            # On hardware the chunks write disjoint slices of one Shared
            # table directly (no copies).
            agT = [dram.tile([P["Npad"], 128], agdt,
                             tag=f"agT{l}", name=f"agT{l}")
                   for l in range(3)]
            ag_ch = [[dram.tile([8 * (AGR[k + 1] - AGR[k]), 128], agdt,
                                addr_space="Shared", tag=f"agch{l}_{k}",
                                name=f"agch{l}_{k}") for k in range(NCH)]
                     for l in range(3)]

            def emit_ag_chunk(l, k):
                r0, r1 = AGR[k], AGR[k + 1]
                nc.gpsimd.collective_compute(
                    "AllGather", mybir.AluOpType.bypass,
                    ins=[ag_in[l][r0:r1, :]], outs=[ag_ch[l][k][:]],
                    replica_groups=RG)

            def emit_ag_copy(l, k):
                import os as _os
                nsp = int(_os.environ.get("K_NSP", "1"))
                r0, r1 = AGR[k], AGR[k + 1]
                rows = 8 * (r1 - r0)
                step = -(-rows // nsp)
                for j in range(0, rows, step):
                    sz = min(step, rows - j)
                    nc.gpsimd.dma_start(
                        agT[l][8 * r0 + j:8 * r0 + j + sz, :],
                        ag_ch[l][k][j:j + sz, :])

            featA = sbp.tile([128, NS], f32, tag="featA")
            featB = sbp.tile([128, NS], f32, tag="featB")
            featOut = sbp.tile([128, NS], f32, tag="featOut")
            featBF = sbp.tile([128, NS], bf16, tag="featBF")
            fsum = featA   # dead by readout time; WAR tracked by Tile
            fmx = featB

            def win_groups():
                grps = []
                w = 0
                while w < NW:
                    grps.append(list(range(w, min(w + G_SIG, NW))))
                    w += G_SIG
                return grps

            # ================= message-passing layers =================
            for l in range(NL):
                featC = [None, featA, featB, featA][l]
                featN = [featA, featB, featA, featB][l]
                dprev = dT[(l + 1) % 2]
                dcur = dT[l % 2]

                gts = {}

                def issue_gather(w):
                    if l > 0 and not NO_GATHER and w < NW:
                        CAPw = KWw[w] * 128
                        gwoff = woff_c[w] * 128
                        g = sbg.tile([128, 1, CAPmax], agdt, tag="gsrc")
                        nc.gpsimd.dma_gather(
                            g[:, :, 0:CAPw], agT[l - 1][:],
                            srcW[:, gwoff // 16:gwoff // 16 + CAPw // 16],
                            CAPw, CAPw, 128, transpose=True,
                            single_packet=False)
                        gts[w] = g

                grps = win_groups()
                for gi, grp in enumerate(grps):
                    if gi == 0:
                        for w in range(0, min(PD, NW)):
                            issue_gather(w)
                    zps = ps_z.tile([128, ZCAP], f32, tag="zps")
                    ewS = sbw.tile([128, ZCAP], f32, tag="ewS")
                    msgEs = {}
                    zoffs = {}
                    zoff = 0

                    # ---- pass 1: messages, z, d-update, msgE ----
                    for w in grp:
                        KW = KWw[w]
                        CAP = KW * 128
                        woff = woff_c[w] * 128
                        zoffs[w] = zoff

                        # prefetch gathers far enough ahead to cover the
                        # AllGather chunk transfer blocking the Pool queue
                        issue_gather(w + PD)
                        gt = gts.pop(w, None)
                        if l > 0:
                            # dst-side term via local one-hot matmul
                            dlb = sbb.tile([128, CAPmax], bf16, tag="dlb")
                            nc.scalar.dma_start(dlb[:, 0:CAP],
                                                t_dstlocB[:, woff:woff + CAP])
                            pY = ps_sm.tile([128, 128], f32, tag="psmall")
                            nc.tensor.matmul(pY[:],
                                             featBF[:, w * WIN:(w + 1) * WIN],
                                             Wb(f"W1b{l}"),
                                             start=True, stop=True)
                            Yt = sbw.tile([128, 128], bf16, tag="Yt")
                            nc.vector.tensor_copy(Yt[:], pY[:])

                        msgEw = sbme.tile([128, CAPmax], bf16, tag="msgEw")
                        msgEs[w] = msgEw

                        s = 0
                        while s * 512 < CAP:
                            sz = min(512, CAP - s * 512)
                            nch = sz // 128
                            soff = woff + s * 512
                            so = s * 512
                            pm = ps_mm.tile([128, 512], f32, tag="pmm")
                            if l == 0:
                                d0s = sbw.tile([3, 512], bf16, tag="d0s")
                                nc.scalar.dma_start(d0s[:, 0:sz],
                                                    t_d0T[:, soff:soff + sz])
                                pdf = ps_mm.tile([128, 512], f32, tag="pmm")
                                nc.tensor.matmul(pdf[:, 0:sz], WeT[:],
                                                 d0s[:, 0:sz],
                                                 start=True, stop=True)
                                dtile = sbw.tile([128, 512], bf16, tag="dtile")
                                nc.scalar.activation(dtile[:, 0:sz],
                                                     pdf[:, 0:sz], AF.Silu,
                                                     bias=V("be"))
                                nc.tensor.matmul(pm[:, 0:sz], Wb("W1c0"),
                                                 dtile[:, 0:sz],
                                                 start=True, stop=True)
                                h1 = sbw.tile([128, 512], bf16, tag="h1")
                                nc.scalar.activation(h1[:, 0:sz], pm[:, 0:sz],
                                                     AF.Silu, bias=V("c0"))
                            else:
                                dtile = sbw.tile([128, 512], bf16, tag="dtile")
                                nc.scalar.dma_start(dtile[:, 0:sz],
                                                    dprev[:, soff:soff + sz])
                                if NO_GATHER:
                                    nc.tensor.matmul(pm[:, 0:sz],
                                                     Wb(f"W1a{l}"),
                                                     dtile[:, 0:sz],
                                                     start=True, stop=False)
                                else:
                                    nc.tensor.matmul(pm[:, 0:sz],
                                                     W8(l) if FP8AG
                                                     else Wb(f"W1a{l}"),
                                                     gt[:, 0, so:so + sz],
                                                     start=True, stop=False)
                                nc.tensor.matmul(pm[:, 0:sz], Wb(f"W1c{l}"),
                                                 dtile[:, 0:sz],
                                                 start=False, stop=False)
                                # one-hot dst expansion
                                oh = sbs.tile([128, 512], bf16, tag="oh")
                                nc.vector.tensor_scalar(
                                    oh[:, 0:sz], dlb[:, so:so + sz],
                                    V("iotaP"), None, op0=OP.is_equal)
                                nc.tensor.matmul(pm[:, 0:sz], Yt[:],
                                                 oh[:, 0:sz],
                                                 start=False, stop=True)
                                h1 = sbw.tile([128, 512], bf16, tag="h1")
                                nc.scalar.activation(h1[:, 0:sz], pm[:, 0:sz],
                                                     AF.Silu, bias=V(f"b1{l}"))
                            pm2 = ps_mm.tile([128, 512], f32, tag="pmm")
                            nc.tensor.matmul(pm2[:, 0:sz], Wb(f"W2{l}"),
                                             h1[:, 0:sz],
                                             start=True, stop=True)
                            msgT = sbw.tile([128, 512], bf16, tag="msgT")
                            nc.scalar.activation(msgT[:, 0:sz], pm2[:, 0:sz],
                                                 AF.Silu, bias=V(f"b2{l}"))
                            if l < NL - 1:
                                dnew = sbw.tile([128, 512], bf16, tag="dnew")
                                nc.vector.tensor_add(dnew[:, 0:sz],
                                                     msgT[:, 0:sz],
                                                     dtile[:, 0:sz])
                                nc.sync.dma_start(dcur[:, soff:soff + sz],
                                                    dnew[:, 0:sz])
                            for k in range(nch):
                                zc = zoff + s * 4 + k
                                nc.tensor.matmul(
                                    zps[:, zc:zc + 1],
                                    msgT[:, k * 128:(k + 1) * 128],
                                    Wb(f"Ws{l}", 1), start=True, stop=True)
                            pme = ps_me.tile([128, 512], bf16, tag="pme")
                            for k in range(nch):
                                nc.tensor.transpose(
                                    pme[:, k * 128:(k + 1) * 128],
                                    msgT[:, k * 128:(k + 1) * 128], identB)
                            nc.vector.tensor_copy(msgEw[:, so:so + sz],
                                                  pme[:, 0:sz])
                            s += 1
                        zoff += KW

                    # ---- batched sigmoid for the group ----
                    nc.scalar.activation(ewS[:, 0:zoff], zps[:, 0:zoff],
                                         AF.Sigmoid, bias=V(f"bs{l}"))

                    # ---- pass 2: segment sums + x into a group tile ----
                    xg = sbw.tile([128, G_SIG * WIN], bf16, tag="xg")
                    for wi, w in enumerate(grp):
                        KW = KWw[w]
                        wsl = slice(w * WIN, (w + 1) * WIN)
                        pmsum = ps_ms.tile([128, WIN], f32, tag="pmsum")
                        for c in range(KW):
                            ci = woff_c[w] + c
                            S = sbs.tile([128, WIN], bf16, tag="S")
                            nc.vector.tensor_scalar(
                                S[:], iotaTb, dstloc[:, ci:ci + 1],
                                ewS[:, zoffs[w] + c:zoffs[w] + c + 1],
                                op0=OP.is_equal, op1=OP.mult)
                            nc.tensor.matmul(
                                pmsum[:],
                                msgEs[w][:, c * 128:(c + 1) * 128],
                                S[:], start=(c == 0), stop=(c == KW - 1))
                        xsl = slice(wi * WIN, (wi + 1) * WIN)
                        if l == 0:
                            nc.vector.tensor_scalar_add(xg[:, xsl], pmsum[:],
                                                        V("emb"))
                        else:
                            nc.vector.tensor_add(xg[:, xsl], pmsum[:],
                                                 featC[:, wsl])

                    # ---- batched update MLP for the whole group ----
                    gsz = len(grp) * WIN
                    gsl = slice(grp[0] * WIN, grp[0] * WIN + gsz)
                    pu = ps_mm.tile([128, 512], f32, tag="pmm")
                    nc.tensor.matmul(pu[:, 0:gsz], Wb(f"Wu1{l}"),
                                     xg[:, 0:gsz], start=True, stop=True)
                    u1 = sbw.tile([128, G_SIG * WIN], bf16, tag="u1")
                    nc.scalar.activation(u1[:, 0:gsz], pu[:, 0:gsz], AF.Silu,
                                         bias=V(f"bu1{l}"))
                    ph = ps_mm.tile([128, 512], f32, tag="pmm")
                    nc.tensor.matmul(ph[:, 0:gsz], Wb(f"Wu2{l}"),
                                     u1[:, 0:gsz], start=True, stop=True)
                    if l == 0:
                        nc.vector.tensor_scalar_add(featN[:, gsl],
                                                    ph[:, 0:gsz],
                                                    V("embPlusBu2"))
                    else:
                        hn = sbw.tile([128, G_SIG * WIN], f32, tag="hn")
                        nc.vector.tensor_scalar_add(hn[:, 0:gsz],
                                                    ph[:, 0:gsz],
                                                    V(f"bu2{l}"))
                        nc.vector.tensor_add(featN[:, gsl], hn[:, 0:gsz],
                                             featC[:, gsl])
                    nc.vector.tensor_copy(featBF[:, gsl], featN[:, gsl])
                    if l < NL - 1:
                        pwbg = ps_me.tile([128, 512], bf16, tag="pme")
                        for wi, w in enumerate(grp):
                            nc.tensor.transpose(
                                pwbg[:, wi * WIN:(wi + 1) * WIN],
                                featBF[:, w * WIN:(w + 1) * WIN], identB)
                        wb = sbw.tile([128, G_SIG * WIN], agdt, tag="wb")
                        nc.vector.tensor_copy(wb[:, 0:gsz], pwbg[:, 0:gsz])
                        for wi, w in enumerate(grp):
                            nc.sync.dma_start(
                                ag_in[l][w * WIN:(w + 1) * WIN, :],
                                wb[:, wi * WIN:(wi + 1) * WIN])
                    else:
                        pn1 = ps_mm.tile([128, 512], f32, tag="pmm")
                        nc.tensor.matmul(pn1[:, 0:gsz], Wb("Wn1"),
                                         featBF[:, gsl],
                                         start=True, stop=True)
                        fo1 = sbw.tile([128, G_SIG * WIN], bf16, tag="fo1")
                        nc.scalar.activation(fo1[:, 0:gsz], pn1[:, 0:gsz],
                                             AF.Silu, bias=V("bn1"))
                        pn2 = ps_mm.tile([128, 512], f32, tag="pmm")
                        nc.tensor.matmul(pn2[:, 0:gsz], Wb("Wn2"),
                                         fo1[:, 0:gsz], start=True, stop=True)
                        nc.vector.tensor_scalar_add(featOut[:, gsl],
                                                    pn2[:, 0:gsz],
                                                    V("bn2"))

                    # chunked AllGather: ship this group's rows now; the
                    # unify-copy for the previous chunk is emitted after the
                    # next chunk's collective so its wait is pre-satisfied.
                    if l < NL - 1 and not NO_AG:
                        wend = grp[-1] + 1              # windows done so far
                        for k in range(NCH):
                            if AGB[k + 1] == wend:
                                emit_ag_chunk(l, k)
                                if k > 0:
                                    emit_ag_copy(l, k - 1)
                                if k == NCH - 1:
                                    emit_ag_copy(l, k)

            # ================= readout =================
            for w in range(NW):
                wsl = slice(w * WIN, (w + 1) * WIN)
                vbR = sbw.tile([1, WIN], f32, tag="vbR")
                nc.sync.dma_start(vbR[:], t_vmask[:, wsl])
                pvb = ps_sm.tile([128, 128], f32, tag="psmall")
                nc.tensor.matmul(pvb[:, 0:WIN], onesP[:], vbR[:],
                                 start=True, stop=True)
                vb = sbw.tile([128, WIN], f32, tag="vb")
                nc.vector.tensor_copy(vb[:], pvb[:, 0:WIN])
                nc.vector.tensor_mul(fsum[:, wsl], featOut[:, wsl], vb[:])
                negm = sbw.tile([128, WIN], f32, tag="negm")
                nc.vector.tensor_scalar(negm[:], vb[:], 1.0, -F32MIN,
                                        op0=OP.subtract, op1=OP.mult)
                nc.vector.tensor_add(fmx[:, wsl], fsum[:, wsl], negm[:])

            rsum8 = sbw.tile([128, 8], f32, tag="rsum8")
            rmax8 = sbw.tile([128, 8], f32, tag="rmax8")
            AX = mybir.AxisListType.X
            for j in range(8):
                nc.vector.tensor_reduce(rsum8[:, j:j + 1],
                                        fsum[:, j * NG:(j + 1) * NG],
                                        axis=AX, op=OP.add)
                nc.vector.tensor_reduce(rmax8[:, j:j + 1],
                                        fmx[:, j * NG:(j + 1) * NG],
                                        axis=AX, op=OP.max)
            rmean8 = sbw.tile([128, 8], f32, tag="rmean8")
            nc.vector.tensor_mul(rmean8[:], rsum8[:], invR8[:])
            rmax8m = sbw.tile([128, 8], f32, tag="rmax8m")
            nc.vector.tensor_mul(rmax8m[:], rmax8[:], presR8[:])

            pq = ps_mm.tile([128, 512], f32, tag="pmm")
            nc.tensor.matmul(pq[:, 0:8], W("Wr1a"), rsum8[:],
                             start=True, stop=False)
            nc.tensor.matmul(pq[:, 0:8], W("Wr1b"), rmean8[:],
                             start=False, stop=False)
            nc.tensor.matmul(pq[:, 0:8], W("Wr1c"), rmax8m[:],
                             start=False, stop=True)
            q = sbw.tile([128, 8], f32, tag="q")
            nc.scalar.activation(q[:], pq[:, 0:8], AF.Relu, bias=V("br1"))
            po = ps_sm.tile([128, 128], f32, tag="psmall")
            nc.tensor.matmul(po[0:1, 0:8], V("Wr2"), q[:],
                             start=True, stop=True)
            ofin = sbw.tile([1, 8], f32, tag="ofin")
            nc.scalar.activation(
                ofin[:], po[0:1, 0:8], AF.Identity,
                bias=wvec[0:1, P["vecs"]["br2"]:P["vecs"]["br2"] + 1])

            rt_in = dram.tile([8, 1], f32, tag="rtin")
            rt_all = dram.tile([G, 1], f32, addr_space="Shared", tag="rtall")
            nc.sync.dma_start(rt_in[:].rearrange("g t -> t g"), ofin[:])
            nc.gpsimd.collective_compute("AllGather", mybir.AluOpType.bypass,
                                         ins=[rt_in[:]], outs=[rt_all[:]],
                                         replica_groups=RG)
            obuf = sbw.tile([1, G], f32, tag="obuf")
            nc.sync.dma_start(obuf[:], rt_all[:].rearrange("g t -> t g"))
            nc.sync.dma_start(t_out.ap().rearrange("g t -> t g"), obuf[:])

    nc.compile()
    return nc


_CACHE = {}


def kernel(**inputs) -> np.ndarray:
    from concourse.bass_utils import run_bass_kernel_spmd

    import os
    params, in_maps = _prep(inputs)
    key = (params["NS"], tuple(params["KWw"]),
           os.environ.get("K_NLAYERS", "4"), os.environ.get("K_SIMAG"),
           os.environ.get("K_FP8AG"), os.environ.get("K_GSIG"),
           os.environ.get("K_SBW"), os.environ.get("K_SBS"),
           os.environ.get("K_SBG"), os.environ.get("K_AGG"),
           os.environ.get("K_AGB"), os.environ.get("K_SBME"),
           os.environ.get("K_SBBB"),
           os.environ.get("K_NO_GATHER"), os.environ.get("K_NO_AG"))
    if key not in _CACHE:
        _CACHE[key] = _build(params)
    nc = _CACHE[key]
    res = run_bass_kernel_spmd(nc, in_maps, list(range(NCORES)))
    return np.asarray(res.results[0]["out"])



# revision 8
# speedup vs baseline: 1.1735x; 1.1735x over previous
"""GNN message-passing (Net3D) Trainium2 kernel, 8-way SPMD.

Strategy
--------
* Sort edges by destination node, shard them across 8 cores by contiguous
  dst ranges (each core's partial segment-sums cover only its own node
  slice); an AllGather of the updated bf16 feature slices feeds the next
  layer's src-side gathers.
* Node layout is padded per *graph* (NG slots per graph, 8 graphs per
  core). Windows of 128 node slots; per-window edge-chunk counts KW_w
  (max over the 8 cores) keep the SPMD instruction stream uniform with
  ~5% edge padding. Ghost edges carry dst_local = -1.
* All matmuls run in bf16 (fp32 is 4x slower on the PE): message MLP,
  update MLP, segment-sum (one-hot selection matrix in bf16), transposes.
* dst-side features are local (edges sorted by dst), so the dst term of
  the message MLP is computed without any gather:
      Yt = featw^T @ W1b  (per window), then
      pm += Yt-contract via a DVE-built one-hot  oh[n,e] = (dstloc==n).
  Only src features are gathered (dma_gather from the AllGather table).
* Soft-edge sigmoids are batched per 4-window group so the ACT
  Silu<->Sigmoid table reload (1.3us each!) happens 2x per group instead
  of 2x per 512 edges.
* Readout: each core reduces its own 8 graphs and runs the readout MLP
  on [*,8]; one tiny [8,1]-per-core AllGather assembles the [64,1] output.
"""

import numpy as np

H = 128
G_FIX = 64
NCORES = 8
WIN = 128
import os as _os0
G_SIG = int(_os0.environ.get("K_GSIG", "4"))
F32MIN = -1.0e30


def _wrap_idxs(idx):
    """int idx [n] -> dma_gather layout [128, n/16] int16 (idx j at
    [j%16, j//16], replicated across the 8 groups of 16 partitions)."""
    n = idx.shape[0]
    assert n % 16 == 0
    t = idx.astype(np.int16).reshape(n // 16, 16).T  # [16, n/16]
    return np.tile(t, (8, 1))


def _prep(inputs):
    """Host-side graph preprocessing -> (params dict, per-core in_maps)."""
    import ml_dtypes
    bf16 = ml_dtypes.bfloat16
    f32 = np.float32

    d0 = np.ascontiguousarray(np.asarray(inputs["d"], dtype=f32))
    src = np.asarray(inputs["src"]).astype(np.int64)
    dst = np.asarray(inputs["dst"]).astype(np.int64)
    ngid = np.asarray(inputs["node_graph_id"]).astype(np.int64)
    G = int(np.asarray(inputs["num_graphs"]))
    N = ngid.shape[0]
    E = src.shape[0]
    assert G == G_FIX and G % NCORES == 0

    cnt = np.bincount(ngid, minlength=G).astype(np.int64)
    NG = int(max(-(-int(cnt.max()) // 32) * 32, 32))   # graph slot size
    NS = G * NG // NCORES                              # node slots per core
    Npad = G * NG
    NW = NS // WIN                                     # windows per core
    assert NS % WIN == 0 and Npad < 32768              # int16 gather idx

    gstart = np.zeros(G, np.int64)
    gstart[1:] = np.cumsum(cnt)[:-1]
    # node -> padded slot; real nodes spread evenly through each graph's
    # NG-slot block (equalizes per-window edge counts)
    pos_in_g = np.arange(N) - gstart[ngid]
    padpos = ngid * NG + (pos_in_g * NG) // np.maximum(cnt[ngid], 1)
    psrc = padpos[src]
    pdst = padpos[dst]

    order = np.argsort(pdst, kind="stable")
    pdst_s = pdst[order]
    psrc_s = psrc[order]
    d0_s = d0[order]

    # per-(core,window) edge counts -> shared per-window chunk counts
    NWG = Npad // WIN
    wstart = np.searchsorted(pdst_s, np.arange(NWG + 1) * WIN)
    wcnt_g = np.diff(wstart).reshape(NCORES, NW)       # [core, window]
    KWw = (-(-wcnt_g.max(axis=0) // 128)).astype(np.int64)  # per window
    KWw = np.maximum(KWw, 1)
    woff_c = np.zeros(NW + 1, np.int64)                # chunk offsets
    woff_c[1:] = np.cumsum(KWw)
    C = int(woff_c[-1])                                # chunks per core
    Ecap = C * 128
    woff_e = woff_c * 128                              # edge offsets
    KWmax = int(KWw.max())
    CAPmax = KWmax * 128

    inv = 1.0 / np.maximum(cnt, 1)
    invcntR = np.tile(inv.astype(f32), (128, 1))
    presentR = np.tile((cnt > 0).astype(f32), (128, 1))

    # AllGather chunking: one chunk per sigmoid window-group; the gather
    # table uses a chunk-major layout (all 8 cores' chunk-k rows are
    # contiguous) so each chunk's collective writes one contiguous range.
    AGG = int(_os0.environ.get("K_AGG", "2"))           # groups per AG chunk
    _agb = _os0.environ.get("K_AGB", "")
    if _agb:
        gbounds = [int(x) for x in _agb.split(",")]
        assert gbounds[0] == 0 and gbounds[-1] == NW
        assert all(b % G_SIG == 0 for b in gbounds[:-1])
    else:
        gbounds = list(range(0, NW, AGG * G_SIG)) + [NW]
    ck = [b * WIN for b in gbounds]                     # local row bounds
    NCHUNK = len(gbounds) - 1

    ck_arr = np.asarray(ck)

    def table_row(p):
        """global padded slot -> chunk-major gather-table row"""
        c = p // NS
        r = p % NS
        k = np.clip(np.searchsorted(ck_arr, r, side="right") - 1,
                    0, NCHUNK - 1)
        sz = ck_arr[k + 1] - ck_arr[k]
        return 8 * ck_arr[k] + c * sz + (r - ck_arr[k])

    # ---- weights / constants (shared by all cores) ----
    emb = np.asarray(inputs["node_embedding"], f32)
    We = np.asarray(inputs["We"], f32)
    be = np.asarray(inputs["be"], f32)
    W1 = np.asarray(inputs["mpW1"], f32)
    b1 = np.asarray(inputs["mpb1"], f32)
    W2 = np.asarray(inputs["mpW2"], f32)
    b2 = np.asarray(inputs["mpb2"], f32)
    Ws = np.asarray(inputs["mpWs"], f32)
    bs = np.asarray(inputs["mpbs"], f32)
    Wu1 = np.asarray(inputs["mpWu1"], f32)
    bu1 = np.asarray(inputs["mpbu1"], f32)
    Wu2 = np.asarray(inputs["mpWu2"], f32)
    bu2 = np.asarray(inputs["mpbu2"], f32)
    Wn1 = np.asarray(inputs["Wn1"], f32)
    bn1 = np.asarray(inputs["bn1"], f32)
    Wn2 = np.asarray(inputs["Wn2"], f32)
    bn2 = np.asarray(inputs["bn2"], f32)
    Wr1 = np.asarray(inputs["Wr1"], f32)
    br1 = np.asarray(inputs["br1"], f32)
    Wr2 = np.asarray(inputs["Wr2"], f32)
    br2 = np.asarray(inputs["br2"], f32)

    c0 = b1[0] + emb @ (W1[0, 0:128] + W1[0, 128:256])
    embPlusBu2 = emb + bu2[0]

    # wmat (f32, readout only); wmatb (bf16, everything else)
    mats = {}
    mcols = []

    def addm(name, arr):
        mats[name] = sum(a.shape[1] for a in mcols)
        mcols.append(np.ascontiguousarray(arr.astype(f32)))

    addm("Wr1a", Wr1[0:128])
    addm("Wr1b", Wr1[128:256])
    addm("Wr1c", Wr1[256:384])
    wmat = np.concatenate(mcols, axis=1)

    matsb = {}
    bcols = []
    _boff = [0]

    def addb(name, arr):
        matsb[name] = _boff[0]
        _boff[0] += arr.shape[1]
        bcols.append(np.ascontiguousarray(arr.astype(bf16)))

    for l in range(4):
        addb(f"W1a{l}", W1[l, 0:128])
        addb(f"W1b{l}", W1[l, 128:256])
        addb(f"W1c{l}", W1[l, 256:384])
        addb(f"W2{l}", W2[l])
        addb(f"Wu1{l}", Wu1[l])
        addb(f"Wu2{l}", Wu2[l])
        addb(f"Ws{l}", Ws[l])                      # [128,1]
        # [I | Ws | 0] so the msg transpose also emits the soft-edge logit
        # z (pad col keeps the 4B PSUM alignment of per-chunk slices)
        addb(f"iWs{l}", np.concatenate(
            [np.eye(128, dtype=f32), Ws[l].astype(f32),
             np.zeros((128, 1), f32)], axis=1))
    addb("Wn1", Wn1)
    addb("Wn2", Wn2)
    addb("identB", np.eye(128, dtype=f32))
    addb("iotaB", np.tile(np.arange(WIN, dtype=f32), (128, 1)))
    wmatb = np.concatenate(bcols, axis=1)

    fp8 = ml_dtypes.float8_e4m3
    wmat8 = np.concatenate(
        [np.ascontiguousarray(W1[l, 0:128].astype(fp8)) for l in range(4)],
        axis=1)

    vecs = {}
    vcols = []

    def addv(name, v):
        vecs[name] = len(vcols)
        vcols.append(np.asarray(v, f32).reshape(128))

    addv("be", be)
    addv("emb", emb)
    addv("c0", c0)
    addv("embPlusBu2", embPlusBu2)
    addv("bn1", bn1)
    addv("bn2", bn2)
    addv("br1", br1)
    addv("Wr2", Wr2[:, 0])
    addv("br2", np.full(128, br2[0], f32))
    addv("iotaP", np.arange(128, dtype=f32))
    for l in range(4):
        addv(f"b1{l}", b1[l])
        addv(f"b2{l}", b2[l])
        addv(f"bs{l}", np.full(128, bs[l, 0], f32))
        addv(f"bu1{l}", bu1[l])
        addv(f"bu2{l}", bu2[l])
    wvec = np.stack(vcols, axis=1)

    base_map = {
        "wmat": wmat,
        "wmat8": wmat8,
        "wmatb": wmatb,
        "wvec": np.ascontiguousarray(wvec),
        "WeT": np.ascontiguousarray(We.astype(bf16)),
    }

    # ---- per-core data ----
    in_maps = []
    for c in range(NCORES):
        lo = c * NS
        src_idx = np.zeros(Ecap, np.int64)
        dloc = np.full(Ecap, -1.0, f32)
        d0T = np.zeros((3, Ecap), bf16)
        for w in range(NW):
            gw = c * NW + w
            a, b = wstart[gw], wstart[gw + 1]
            k = b - a
            off = int(woff_e[w])
            assert k <= KWw[w] * 128
            src_idx[off:off + k] = table_row(psrc_s[a:b])
            dloc[off:off + k] = (pdst_s[a:b] - (lo + w * WIN)).astype(f32)
            d0T[:, off:off + k] = d0_s[a:b].T
        occ = np.zeros(Npad, f32)
        occ[padpos] = 1.0
        vmask = occ[lo:lo + NS].reshape(1, NS)

        m = dict(base_map)
        m.update({
            "srcW": _wrap_idxs(src_idx),
            "dstloc": np.ascontiguousarray(dloc.reshape(C, 128).T),
            "dstlocB": np.ascontiguousarray(
                np.tile(dloc.astype(bf16), (128, 1))),
            "d0T": d0T,
            "vmaskR": vmask,
            "invcnt8": np.ascontiguousarray(invcntR[:, c * 8:(c + 1) * 8]),
            "present8": np.ascontiguousarray(presentR[:, c * 8:(c + 1) * 8]),
        })
        in_maps.append(m)

    params = dict(NG=NG, NS=NS, Npad=Npad, NW=NW, C=C, Ecap=Ecap, G=G,
                  KWw=[int(x) for x in KWw], woff_c=[int(x) for x in woff_c],
                  KWmax=KWmax, CAPmax=CAPmax, AGR=ck, NCHUNK=NCHUNK,
                  AGB=gbounds,
                  mats=mats, vecs=vecs, matsb=matsb,
                  wmat_cols=wmat.shape[1], wvec_cols=wvec.shape[1],
                  wmatb_cols=wmatb.shape[1])
    return params, in_maps


def _build(P):
    import concourse.bacc as bacc
    import concourse.mybir as mybir
    import concourse.tile as tile

    f32 = mybir.dt.float32
    bf16 = mybir.dt.bfloat16
    i16 = mybir.dt.int16
    AF = mybir.ActivationFunctionType
    OP = mybir.AluOpType
    RG = [list(range(NCORES))]

    NS, NW, C, Ecap, G, NG = (P["NS"], P["NW"], P["C"], P["Ecap"],
                              P["G"], P["NG"])
    KWw, woff_c = P["KWw"], P["woff_c"]
    KWmax, CAPmax = P["KWmax"], P["CAPmax"]
    ZCAP = G_SIG * KWmax       # z cols per sigmoid group (upper bound)

    import os
    NL = int(os.environ.get("K_NLAYERS", "4"))
    FP8AG = os.environ.get("K_FP8AG") == "1"
    AGG = int(os.environ.get("K_AGG", "2"))
    AGB = P["AGB"]
    PD = (AGG + 1) * G_SIG           # gather prefetch distance (windows)
    NO_GATHER = os.environ.get("K_NO_GATHER") == "1"
    NO_AG = os.environ.get("K_NO_AG") == "1"
    nc = bacc.Bacc("TRN2", target_bir_lowering=False, debug=False,
                   num_devices=NCORES)

    t_wmat = nc.dram_tensor("wmat", [128, P["wmat_cols"]], f32, kind="ExternalInput")
    t_wvec = nc.dram_tensor("wvec", [128, P["wvec_cols"]], f32, kind="ExternalInput")
    t_wmatb = nc.dram_tensor("wmatb", [128, P["wmatb_cols"]], bf16, kind="ExternalInput")
    fp8 = mybir.dt.float8e4
    agdt = fp8 if FP8AG else bf16
    t_wmat8 = nc.dram_tensor("wmat8", [128, 512], fp8, kind="ExternalInput")
    t_We = nc.dram_tensor("WeT", [3, 128], bf16, kind="ExternalInput")
    t_inv8 = nc.dram_tensor("invcnt8", [128, 8], f32, kind="ExternalInput")
    t_pres8 = nc.dram_tensor("present8", [128, 8], f32, kind="ExternalInput")
    t_srcW = nc.dram_tensor("srcW", [128, Ecap // 16], i16, kind="ExternalInput")
    t_dstloc = nc.dram_tensor("dstloc", [128, C], f32, kind="ExternalInput")
    t_dstlocB = nc.dram_tensor("dstlocB", [128, Ecap], bf16, kind="ExternalInput")
    t_d0T = nc.dram_tensor("d0T", [3, Ecap], bf16, kind="ExternalInput")
    t_vmask = nc.dram_tensor("vmaskR", [1, NS], f32, kind="ExternalInput")
    t_out = nc.dram_tensor("out", [G, 1], f32, kind="ExternalOutput")

    with tile.TileContext(nc) as tc:
        with (
            tc.tile_pool(name="sbc", bufs=1) as sbc,
            tc.tile_pool(name="sbp", bufs=1) as sbp,      # persistent feats
            tc.tile_pool(name="sbg", bufs=int(os.environ.get("K_SBG", "15"))) as sbg,      # gather landing
            tc.tile_pool(name="sbb", bufs=int(os.environ.get("K_SBBB", "3"))) as sbb,      # dstlocB stream
            tc.tile_pool(name="sbw", bufs=int(os.environ.get("K_SBW", "3"))) as sbw,      # working tiles
            tc.tile_pool(name="sbme", bufs=int(os.environ.get("K_SBME", "5"))) as sbme,    # per-window msgE
            tc.tile_pool(name="sbs", bufs=int(os.environ.get("K_SBS", "4"))) as sbs,      # S / one-hot tiles
            tc.tile_pool(name="ps_mm", bufs=3, space="PSUM") as ps_mm,
            tc.tile_pool(name="ps_me", bufs=1, space="PSUM") as ps_me,
            tc.tile_pool(name="ps_ms", bufs=2, space="PSUM") as ps_ms,
            tc.tile_pool(name="ps_sm", bufs=1, space="PSUM") as ps_sm,
            tc.tile_pool(name="dram", bufs=1, space="DRAM") as dram,
        ):
            # ---- constants ----
            wmat = sbc.tile([128, P["wmat_cols"]], f32, tag="wmat")
            nc.sync.dma_start(wmat[:], t_wmat[:])
            wvec = sbc.tile([128, P["wvec_cols"]], f32, tag="wvec")
            nc.sync.dma_start(wvec[:], t_wvec[:])
            wmatb = sbc.tile([128, P["wmatb_cols"]], bf16, tag="wmatb")
            nc.sync.dma_start(wmatb[:], t_wmatb[:])
            wmat8 = sbc.tile([128, 512], fp8, tag="wmat8")
            nc.sync.dma_start(wmat8[:], t_wmat8[:])
            WeT = sbc.tile([3, 128], bf16, tag="WeT")
            nc.sync.dma_start(WeT[:], t_We[:])
            invR8 = sbc.tile([128, 8], f32, tag="invR8")
            nc.sync.dma_start(invR8[:], t_inv8[:])
            presR8 = sbc.tile([128, 8], f32, tag="presR8")
            nc.sync.dma_start(presR8[:], t_pres8[:])
            srcW = sbc.tile([128, Ecap // 16], i16, tag="srcW")
            nc.sync.dma_start(srcW[:], t_srcW[:])
            dstloc = sbc.tile([128, C], f32, tag="dstloc")
            nc.sync.dma_start(dstloc[:], t_dstloc[:])
            onesP = sbc.tile([1, 128], f32, tag="onesP")
            nc.vector.memset(onesP[:], 1.0)

            def W(name):
                o = P["mats"][name]
                return wmat[:, o:o + 128]

            def Wb(name, n=128):
                o = P["matsb"][name]
                return wmatb[:, o:o + n]

            def V(name):
                o = P["vecs"][name]
                return wvec[:, o:o + 1]

            def W8(l):
                return wmat8[:, l * 128:(l + 1) * 128]

            identB = Wb("identB")
            iotaTb = Wb("iotaB", WIN)

            tc.strict_bb_all_engine_barrier()

            # ---- DRAM work buffers ----
            dT = [dram.tile([128, Ecap], bf16, tag=f"dT{i}", name=f"dT{i}")
                  for i in range(2)]
            ag_in = [dram.tile([NS, 128], agdt, tag=f"agin{l}",
                               name=f"agin{l}") for l in range(3)]
            NCH = P["NCHUNK"]
            AGR = P["AGR"]
            # CoreSim only supports a single writer per Shared tensor, so the
            # local-sim path (K_SIMAG=1) routes each AllGather chunk through
            # its own Shared tensor plus a unify copy on the ACT HWDGE queue.
            # On hardware the chunks write disjoint slices of one Shared
            # table directly (no copies).
            agT = [dram.tile([P["Npad"], 128], agdt,
                             tag=f"agT{l}", name=f"agT{l}")
                   for l in range(3)]
            ag_ch = [[dram.tile([8 * (AGR[k + 1] - AGR[k]), 128], agdt,
                                addr_space="Shared", tag=f"agch{l}_{k}",
                                name=f"agch{l}_{k}") for k in range(NCH)]
                     for l in range(3)]

            def emit_ag_chunk(l, k):
                r0, r1 = AGR[k], AGR[k + 1]
                nc.gpsimd.collective_compute(
                    "AllGather", mybir.AluOpType.bypass,
                    ins=[ag_in[l][r0:r1, :]], outs=[ag_ch[l][k][:]],
                    replica_groups=RG)

            def emit_ag_copy(l, k):
                import os as _os
                nsp = int(_os.environ.get("K_NSP", "1"))
                r0, r1 = AGR[k], AGR[k + 1]
                rows = 8 * (r1 - r0)
                step = -(-rows // nsp)
                for j in range(0, rows, step):
                    sz = min(step, rows - j)
                    nc.gpsimd.dma_start(
                        agT[l][8 * r0 + j:8 * r0 + j + sz, :],
                        ag_ch[l][k][j:j + sz, :])

            featA = sbp.tile([128, NS], f32, tag="featA")
            featB = sbp.tile([128, NS], f32, tag="featB")
            featOut = sbp.tile([128, NS], f32, tag="featOut")
            featBF = sbp.tile([128, NS], bf16, tag="featBF")
            fsum = featA   # dead by readout time; WAR tracked by Tile
            fmx = featB

            def win_groups():
                grps = []
                w = 0
                while w < NW:
                    grps.append(list(range(w, min(w + G_SIG, NW))))
                    w += G_SIG
                return grps

            # ================= message-passing layers =================
            for l in range(NL):
                featC = [None, featA, featB, featA][l]
                featN = [featA, featB, featA, featB][l]
                dprev = dT[(l + 1) % 2]
                dcur = dT[l % 2]

                gts = {}

                def issue_gather(w):
                    if l > 0 and not NO_GATHER and w < NW:
                        CAPw = KWw[w] * 128
                        gwoff = woff_c[w] * 128
                        g = sbg.tile([128, 1, CAPmax], agdt, tag="gsrc")
                        nc.gpsimd.dma_gather(
                            g[:, :, 0:CAPw], agT[l - 1][:],
                            srcW[:, gwoff // 16:gwoff // 16 + CAPw // 16],
                            CAPw, CAPw, 128, transpose=True,
                            single_packet=False)
                        gts[w] = g

                grps = win_groups()
                for gi, grp in enumerate(grps):
                    if gi == 0:
                        for w in range(0, min(PD, NW)):
                            issue_gather(w)
                    zg = sbw.tile([128, ZCAP], bf16, tag="zg")
                    ewS = sbw.tile([128, ZCAP], f32, tag="ewS")
                    msgEs = {}
                    zoffs = {}
                    zoff = 0

                    # ---- pass 1: messages, z, d-update, msgE ----
                    for w in grp:
                        KW = KWw[w]
                        CAP = KW * 128
                        woff = woff_c[w] * 128
                        zoffs[w] = zoff

                        # prefetch gathers far enough ahead to cover the
                        # AllGather chunk transfer blocking the Pool queue
                        issue_gather(w + PD)
                        gt = gts.pop(w, None)
                        if l > 0:
                            # dst-side term via local one-hot matmul
                            dlb = sbb.tile([128, CAPmax], bf16, tag="dlb")
                            nc.scalar.dma_start(dlb[:, 0:CAP],
                                                t_dstlocB[:, woff:woff + CAP])
                            pY = ps_sm.tile([128, 128], f32, tag="psmall")
                            nc.tensor.matmul(pY[:],
                                             featBF[:, w * WIN:(w + 1) * WIN],
                                             Wb(f"W1b{l}"),
                                             start=True, stop=True)
                            Yt = sbw.tile([128, 128], bf16, tag="Yt")
                            nc.vector.tensor_copy(Yt[:], pY[:])

                        msgEw = sbme.tile([128, CAPmax], bf16, tag="msgEw")
                        msgEs[w] = msgEw

                        s = 0
                        while s * 512 < CAP:
                            sz = min(512, CAP - s * 512)
                            nch = sz // 128
                            soff = woff + s * 512
                            so = s * 512
                            pm = ps_mm.tile([128, 512], f32, tag="pmm")
                            if l == 0:
                                d0s = sbw.tile([3, 512], bf16, tag="d0s")
                                nc.scalar.dma_start(d0s[:, 0:sz],
                                                    t_d0T[:, soff:soff + sz])
                                pdf = ps_mm.tile([128, 512], f32, tag="pmm")
                                nc.tensor.matmul(pdf[:, 0:sz], WeT[:],
                                                 d0s[:, 0:sz],
                                                 start=True, stop=True)
                                dtile = sbw.tile([128, 512], bf16, tag="dtile")
                                nc.scalar.activation(dtile[:, 0:sz],
                                                     pdf[:, 0:sz], AF.Silu,
                                                     bias=V("be"))
                                nc.tensor.matmul(pm[:, 0:sz], Wb("W1c0"),
                                                 dtile[:, 0:sz],
                                                 start=True, stop=True)
                                h1 = sbw.tile([128, 512], bf16, tag="h1")
                                nc.scalar.activation(h1[:, 0:sz], pm[:, 0:sz],
                                                     AF.Silu, bias=V("c0"))
                            else:
                                dtile = sbw.tile([128, 512], bf16, tag="dtile")
                                nc.scalar.dma_start(dtile[:, 0:sz],
                                                    dprev[:, soff:soff + sz])
                                if NO_GATHER:
                                    nc.tensor.matmul(pm[:, 0:sz],
                                                     Wb(f"W1a{l}"),
                                                     dtile[:, 0:sz],
                                                     start=True, stop=False)
                                else:
                                    nc.tensor.matmul(pm[:, 0:sz],
                                                     W8(l) if FP8AG
                                                     else Wb(f"W1a{l}"),
                                                     gt[:, 0, so:so + sz],
                                                     start=True, stop=False)
                                nc.tensor.matmul(pm[:, 0:sz], Wb(f"W1c{l}"),
                                                 dtile[:, 0:sz],
                                                 start=False, stop=False)
                                # one-hot dst expansion
                                oh = sbs.tile([128, 512], bf16, tag="oh")
                                nc.vector.tensor_scalar(
                                    oh[:, 0:sz], dlb[:, so:so + sz],
                                    V("iotaP"), None, op0=OP.is_equal)
                                nc.tensor.matmul(pm[:, 0:sz], Yt[:],
                                                 oh[:, 0:sz],
                                                 start=False, stop=True)
                                h1 = sbw.tile([128, 512], bf16, tag="h1")
                                nc.scalar.activation(h1[:, 0:sz], pm[:, 0:sz],
                                                     AF.Silu, bias=V(f"b1{l}"))
                            pm2 = ps_mm.tile([128, 512], f32, tag="pmm")
                            nc.tensor.matmul(pm2[:, 0:sz], Wb(f"W2{l}"),
                                             h1[:, 0:sz],
                                             start=True, stop=True)
                            msgT = sbw.tile([128, 512], bf16, tag="msgT")
                            nc.scalar.activation(msgT[:, 0:sz], pm2[:, 0:sz],
                                                 AF.Silu, bias=V(f"b2{l}"))
                            if l < NL - 1:
                                dnew = sbw.tile([128, 512], bf16, tag="dnew")
                                nc.vector.tensor_add(dnew[:, 0:sz],
                                                     msgT[:, 0:sz],
                                                     dtile[:, 0:sz])
                                nc.sync.dma_start(dcur[:, soff:soff + sz],
                                                    dnew[:, 0:sz])
                            # fused transpose+z: rhs = [I | Ws] so column 128
                            # of each chunk's transpose is the soft-edge logit
                            pme = ps_me.tile([128, 4, 130], bf16, tag="pme")
                            for k in range(nch):
                                nc.tensor.transpose(
                                    pme[:, k, :],
                                    msgT[:, k * 128:(k + 1) * 128],
                                    Wb(f"iWs{l}", 130))
                            nc.vector.tensor_copy(
                                msgEw[:, so:so + sz].rearrange(
                                    "p (c h) -> p c h", c=nch),
                                pme[:, 0:nch, 0:128])
                            zc0 = zoff + s * 4
                            nc.vector.tensor_copy(
                                zg[:, zc0:zc0 + nch],
                                pme[:, 0:nch, 128:129].rearrange(
                                    "p c o -> p (c o)"))
                            s += 1
                        zoff += KW

                    # ---- batched sigmoid for the group ----
                    nc.scalar.activation(ewS[:, 0:zoff], zg[:, 0:zoff],
                                         AF.Sigmoid, bias=V(f"bs{l}"))

                    # ---- pass 2: segment sums + x into a group tile ----
                    xg = sbw.tile([128, G_SIG * WIN], bf16, tag="xg")
                    for wi, w in enumerate(grp):
                        KW = KWw[w]
                        wsl = slice(w * WIN, (w + 1) * WIN)
                        pmsum = ps_ms.tile([128, WIN], f32, tag="pmsum")
                        for c in range(KW):
                            ci = woff_c[w] + c
                            S = sbs.tile([128, WIN], bf16, tag="S")
                            nc.vector.tensor_scalar(
                                S[:], iotaTb, dstloc[:, ci:ci + 1],
                                ewS[:, zoffs[w] + c:zoffs[w] + c + 1],
                                op0=OP.is_equal, op1=OP.mult)
                            nc.tensor.matmul(
                                pmsum[:],
                                msgEs[w][:, c * 128:(c + 1) * 128],
                                S[:], start=(c == 0), stop=(c == KW - 1))
                        xsl = slice(wi * WIN, (wi + 1) * WIN)
                        if l == 0:
                            nc.vector.tensor_scalar_add(xg[:, xsl], pmsum[:],
                                                        V("emb"))
                        else:
                            nc.vector.tensor_add(xg[:, xsl], pmsum[:],
                                                 featC[:, wsl])

                    # ---- batched update MLP for the whole group ----
                    gsz = len(grp) * WIN
                    gsl = slice(grp[0] * WIN, grp[0] * WIN + gsz)
                    pu = ps_mm.tile([128, 512], f32, tag="pmm")
                    nc.tensor.matmul(pu[:, 0:gsz], Wb(f"Wu1{l}"),
                                     xg[:, 0:gsz], start=True, stop=True)
                    u1 = sbw.tile([128, G_SIG * WIN], bf16, tag="u1")
                    nc.scalar.activation(u1[:, 0:gsz], pu[:, 0:gsz], AF.Silu,
                                         bias=V(f"bu1{l}"))
                    ph = ps_mm.tile([128, 512], f32, tag="pmm")
                    nc.tensor.matmul(ph[:, 0:gsz], Wb(f"Wu2{l}"),
                                     u1[:, 0:gsz], start=True, stop=True)
                    if l == 0:
                        nc.vector.tensor_scalar_add(featN[:, gsl],
                                                    ph[:, 0:gsz],
                                                    V("embPlusBu2"))
                    else:
                        hn = sbw.tile([128, G_SIG * WIN], f32, tag="hn")
                        nc.vector.tensor_scalar_add(hn[:, 0:gsz],
                                                    ph[:, 0:gsz],
                                                    V(f"bu2{l}"))
                        nc.vector.tensor_add(featN[:, gsl], hn[:, 0:gsz],
                                             featC[:, gsl])
                    nc.vector.tensor_copy(featBF[:, gsl], featN[:, gsl])
                    if l < NL - 1:
                        pwbg = ps_me.tile([128, 512], bf16, tag="pme")
                        for wi, w in enumerate(grp):
                            nc.tensor.transpose(
                                pwbg[:, wi * WIN:(wi + 1) * WIN],
                                featBF[:, w * WIN:(w + 1) * WIN], identB)
                        wb = sbw.tile([128, G_SIG * WIN], agdt, tag="wb")
                        nc.vector.tensor_copy(wb[:, 0:gsz], pwbg[:, 0:gsz])
                        for wi, w in enumerate(grp):
                            nc.sync.dma_start(
                                ag_in[l][w * WIN:(w + 1) * WIN, :],
                                wb[:, wi * WIN:(wi + 1) * WIN])
                    else:
                        pn1 = ps_mm.tile([128, 512], f32, tag="pmm")
                        nc.tensor.matmul(pn1[:, 0:gsz], Wb("Wn1"),
                                         featBF[:, gsl],
                                         start=True, stop=True)
                        fo1 = sbw.tile([128, G_SIG * WIN], bf16, tag="fo1")
                        nc.scalar.activation(fo1[:, 0:gsz], pn1[:, 0:gsz],
                                             AF.Silu, bias=V("bn1"))
                        pn2 = ps_mm.tile([128, 512], f32, tag="pmm")
                        nc.tensor.matmul(pn2[:, 0:gsz], Wb("Wn2"),
                                         fo1[:, 0:gsz], start=True, stop=True)
                        nc.vector.tensor_scalar_add(featOut[:, gsl],
                                                    pn2[:, 0:gsz],
                                                    V("bn2"))

                    # chunked AllGather: ship this group's rows now; the
                    # unify-copy for the previous chunk is emitted after the
                    # next chunk's collective so its wait is pre-satisfied.
                    if l < NL - 1 and not NO_AG:
                        wend = grp[-1] + 1              # windows done so far
                        for k in range(NCH):
                            if AGB[k + 1] == wend:
                                emit_ag_chunk(l, k)
                                if k > 0:
                                    emit_ag_copy(l, k - 1)
                                if k == NCH - 1:
                                    emit_ag_copy(l, k)

            # ================= readout =================
            for w in range(NW):
                wsl = slice(w * WIN, (w + 1) * WIN)
                vbR = sbw.tile([1, WIN], f32, tag="vbR")
                nc.sync.dma_start(vbR[:], t_vmask[:, wsl])
                pvb = ps_sm.tile([128, 128], f32, tag="psmall")
                nc.tensor.matmul(pvb[:, 0:WIN], onesP[:], vbR[:],
                                 start=True, stop=True)
                vb = sbw.tile([128, WIN], f32, tag="vb")
                nc.vector.tensor_copy(vb[:], pvb[:, 0:WIN])
                nc.vector.tensor_mul(fsum[:, wsl], featOut[:, wsl], vb[:])
                negm = sbw.tile([128, WIN], f32, tag="negm")
                nc.vector.tensor_scalar(negm[:], vb[:], 1.0, -F32MIN,
                                        op0=OP.subtract, op1=OP.mult)
                nc.vector.tensor_add(fmx[:, wsl], fsum[:, wsl], negm[:])

            rsum8 = sbw.tile([128, 8], f32, tag="rsum8")
            rmax8 = sbw.tile([128, 8], f32, tag="rmax8")
            AX = mybir.AxisListType.X
            for j in range(8):
                nc.vector.tensor_reduce(rsum8[:, j:j + 1],
                                        fsum[:, j * NG:(j + 1) * NG],
                                        axis=AX, op=OP.add)
                nc.vector.tensor_reduce(rmax8[:, j:j + 1],
                                        fmx[:, j * NG:(j + 1) * NG],
                                        axis=AX, op=OP.max)
            rmean8 = sbw.tile([128, 8], f32, tag="rmean8")
            nc.vector.tensor_mul(rmean8[:], rsum8[:], invR8[:])
            rmax8m = sbw.tile([128, 8], f32, tag="rmax8m")
            nc.vector.tensor_mul(rmax8m[:], rmax8[:], presR8[:])

            pq = ps_mm.tile([128, 512], f32, tag="pmm")
            nc.tensor.matmul(pq[:, 0:8], W("Wr1a"), rsum8[:],
                             start=True, stop=False)
            nc.tensor.matmul(pq[:, 0:8], W("Wr1b"), rmean8[:],
                             start=False, stop=False)
            nc.tensor.matmul(pq[:, 0:8], W("Wr1c"), rmax8m[:],
                             start=False, stop=True)
            q = sbw.tile([128, 8], f32, tag="q")
            nc.scalar.activation(q[:], pq[:, 0:8], AF.Relu, bias=V("br1"))
            po = ps_sm.tile([128, 128], f32, tag="psmall")
            nc.tensor.matmul(po[0:1, 0:8], V("Wr2"), q[:],
                             start=True, stop=True)
            ofin = sbw.tile([1, 8], f32, tag="ofin")
            nc.scalar.activation(
                ofin[:], po[0:1, 0:8], AF.Identity,
                bias=wvec[0:1, P["vecs"]["br2"]:P["vecs"]["br2"] + 1])

            rt_in = dram.tile([8, 1], f32, tag="rtin")
            rt_all = dram.tile([G, 1], f32, addr_space="Shared", tag="rtall")
            nc.sync.dma_start(rt_in[:].rearrange("g t -> t g"), ofin[:])
            nc.gpsimd.collective_compute("AllGather", mybir.AluOpType.bypass,
                                         ins=[rt_in[:]], outs=[rt_all[:]],
                                         replica_groups=RG)
            obuf = sbw.tile([1, G], f32, tag="obuf")
            nc.sync.dma_start(obuf[:], rt_all[:].rearrange("g t -> t g"))
            nc.sync.dma_start(t_out.ap().rearrange("g t -> t g"), obuf[:])

    nc.compile()
    return nc


_CACHE = {}


def kernel(**inputs) -> np.ndarray:
    from concourse.bass_utils import run_bass_kernel_spmd

    import os
    params, in_maps = _prep(inputs)
    key = (params["NS"], tuple(params["KWw"]),
           os.environ.get("K_NLAYERS", "4"), os.environ.get("K_SIMAG"),
           os.environ.get("K_FP8AG"), os.environ.get("K_GSIG"),
           os.environ.get("K_SBW"), os.environ.get("K_SBS"),
           os.environ.get("K_SBG"), os.environ.get("K_AGG"),
           os.environ.get("K_AGB"), os.environ.get("K_SBME"),
           os.environ.get("K_SBBB"),
           os.environ.get("K_NO_GATHER"), os.environ.get("K_NO_AG"))
    if key not in _CACHE:
        _CACHE[key] = _build(params)
    nc = _CACHE[key]
    res = run_bass_kernel_spmd(nc, in_maps, list(range(NCORES)))
    return np.asarray(res.results[0]["out"])

